# revision 30
# baseline (speedup 1.0000x reference)
"""Modulated Conv2D (StyleGAN2-style) Trainium2 Bass kernel.

Problem shapes (hardcoded):
  x: [16, 256, 64, 64] f32    y: [16, 512] f32
  weights: [256, 256, 3, 3]   bias: [256]
  style_w: [256, 512]         style_b: [256]
  out: [16, 256, 64, 64] f32

Formulation: 1-D Winograd F(2,3) along H + direct 3-tap conv along W,
with the per-sample style modulation folded into the weights:
  style[b,i] = y[b] @ style_w[i] + style_b[i]        (tiny PE matmul)
  U[kh,kx,i,o] = sum_ky G[kh,ky] * w[o,i,ky,kx]      (host precompute, bf16)
  um[b][i,kh*3+kx,o] = U[i,kh*3+kx,o] * style[b,i]   (DVE)
  V0 = d0-d2; V1 = d1+d2; V2 = d2-d1; V3 = d1-d3     (DVE; dk = padded x
                                                      rows 2m+k, tile-row m)
  M[kh][o,m,c] = sum_{i,kx} um[kh,kx,i,o] V[kh][i,m,c+kx]   (PE, f32 PSUM)
  out[2m+0] = (M0+M1+M2) / wstd + bias               (DVE adds + scalar act)
  out[2m+1] = (M1-M2-M3) / wstd + bias
  wstd[b,o] = sqrt(sum_i W2[i,o] * style[b,i]^2 + eps),
  W2[i,o] = sum_kk w[o,i,kk]^2                       (host precompute)

This does the conv in 24 accumulating matmuls per 8-tile-row chunk
instead of direct conv's 36 (1.5x fewer PE cycles); the Winograd
transforms run on DVE/scalar fully under the PE stream.

Host-side layout prep (layout/dtype packing + static weight transform):
  - ut = G-transformed weights [Cin, 12, Cout] bf16 (lhsT-ready).
  - w2 = per-(i,o) weight square sums f32 (demod path operand).
  - swt/yT packed bf16 so style is a [512]-contraction PE matmul.
  - x zero-padded to [66,66] bf16, matmul-ready tiles.
  - device output bf16 with even/odd output rows in separate planes;
    host interleaves + upcasts (budget 2e-2, this scheme ~4.5e-3).

Sharding: data-parallel over batch, 2 samples per core across 8 cores.

Engines: PE does style + sigma + conv matmuls; DVE squares style,
modulates U, computes the V input transform and the M->Y output
transform; Scalar copies M1 out of PSUM and applies 1/wstd + bias
(f32->bf16); scalar HW DMA ring loads weights, sync ring x + stores.
"""

import numpy as np
import ml_dtypes

import concourse.bass as bass
import concourse.tile as tile
from concourse import bacc, mybir
from concourse import bass_utils

EPS = 1e-8
P = 128
B_LOC = 2          # samples per core
B_FULL = 16
CIN, COUT = 256, 256
NI, NO = CIN // P, COUT // P   # 2, 2
S = 512
NS = S // P        # 4 style contraction blocks
KH, KX = 4, 3      # winograd points along H, direct taps along W
KK = KH * KX       # 12 lhsT planes
KA = 6             # first kk-chunk of the ut loads
H = W = 64
HP, WP = H + 2, W + 2  # zero-padded image
TR = H // 2        # 32 winograd tile-rows
N_CORES = 8
ROWS_A = 34        # rows in the first half of each x tile load

F32 = mybir.dt.float32
BF16 = mybir.dt.bfloat16
AF = mybir.ActivationFunctionType


def _chain(instrs, reason):
    """Force program order on one engine (guides the tile scheduler)."""
    for a, b in zip(instrs[1:], instrs[:-1]):
        bass._add_dep_helper(a.ins, b.ins, sync=False, reason=reason)


def build_conv2dmod(nc):
    xp = nc.dram_tensor("xp", [B_LOC, CIN, HP, WP], BF16, kind="ExternalInput")
    ut = nc.dram_tensor("ut", [CIN, KK, COUT], BF16, kind="ExternalInput")
    # host-prepacked [sp, sb, i+b]: style_w.T columns then y.T columns
    swt = nc.dram_tensor("swt", [P, NS, CIN + B_LOC], BF16, kind="ExternalInput")
    # host-prepacked [128, 5] f32: bias cols (2), style_b cols (2), eps (1)
    cst = nc.dram_tensor("cst", [P, 2 * NO + 1], F32, kind="ExternalInput")
    # host-prepacked W2T [i_part, it, o] f32
    w2 = nc.dram_tensor("w2", [P, NI, COUT], F32, kind="ExternalInput")
    # even/odd output rows in separate planes; host interleaves
    out = nc.dram_tensor("out", [B_LOC, COUT, 2, TR, W], BF16,
                         kind="ExternalOutput")

    with tile.TileContext(nc) as tc:
        with (
            tc.tile_pool(name="consts", bufs=1) as consts,
            tc.tile_pool(name="temps", bufs=2) as temps,
            tc.tile_pool(name="um_pool", bufs=1) as um_pool,
            tc.tile_pool(name="xs_pool", bufs=1) as xs_pool,
            tc.tile_pool(name="v_pool", bufs=1) as v_pool,
            tc.tile_pool(name="out_pool", bufs=3) as out_pool,
            tc.tile_pool(name="psum", bufs=2, space="PSUM") as psum,
        ):
            # ---------------- tiles ----------------
            swt_t = consts.tile([P, NS, CIN + B_LOC], BF16)
            ut_t = [consts.tile([P, KK, COUT], BF16, name=f"ut{i}", tag=f"ut{i}")
                    for i in range(NI)]
            w2_t = consts.tile([P, NI, COUT], F32)
            cst_t = consts.tile([P, 2 * NO + 1], F32)
            xs = {}
            vt = {}
            for s in range(B_LOC):
                for cb in range(NI):
                    xs[(s, cb)] = xs_pool.tile(
                        [P, HP, WP], BF16, name=f"xs{s}_{cb}", tag=f"xs{s}_{cb}")
                    vt[(s, cb)] = v_pool.tile(
                        [P, KH, TR, WP], BF16, name=f"vt{s}_{cb}",
                        tag=f"vt{s}_{cb}")
            um = {}
            for s in range(B_LOC):
                for it in range(NI):
                    um[(s, it)] = um_pool.tile(
                        [P, KK, COUT], BF16, name=f"um{s}_{it}",
                        tag=f"um{s}_{it}")

            def bias_ap(ot):
                return cst_t[:, ot:ot + 1]

            def style_b_ap(it):
                return cst_t[:, NO + it:NO + it + 1]

            eps_ap = cst_t[:, 2 * NO:2 * NO + 1]

            # pre-warm the ACT function table that Sqrt lives in; must be
            # dependency-free so it runs (and the async table load fires)
            # before anything else on the scalar engine
            warm_src = consts.tile([P, 1], F32)
            nc.gpsimd.memset(warm_src[:], EPS)
            lafs_warm = consts.tile([P, 1], F32)
            warm_i = nc.scalar.activation(lafs_warm[:], warm_src[:], AF.Sqrt)

            # PE HAM warm-up: ~3.5us of zeroed junk matmuls during the DMA
            # prologue so the PE clock gate is already at 8/8 (2.4 GHz) when
            # the real conv stream starts. The junk psum tiles borrow the
            # style tags ahead of their first real use so the conv chunks'
            # kh0/kh1 rotation parity is untouched.
            jw = consts.tile([P, P], BF16)
            jx = consts.tile([P, 8, W], BF16)
            nc.gpsimd.memset(jw[:], 0.0)
            nc.gpsimd.memset(jx[:], 0.0)
            for r in range(8):
                pj = psum.tile([P, 8, W], F32, name=f"jp{r}",
                               tag=f"kh{2 + r % 2}")
                nc.tensor.matmul(pj[:], jw[:], jx[:], start=True, stop=True)

            # ------------- DMA rings, ordered by when they gate compute -------
            # the two HWDGE rings share ~330 GB/s of HBM bandwidth; order
            # both by when the bytes gate compute (it0 weights + x h0 first)
            scalar_ring = [
                nc.scalar.dma_start(ut_t[0][:, 0:KA, :], ut.ap()[0:P, 0:KA]),
                nc.scalar.dma_start(swt_t[:], swt.ap()),
                nc.scalar.dma_start(ut_t[0][:, KA:KK, :], ut.ap()[0:P, KA:KK]),
            ]
            _chain([warm_i] + scalar_ring, "warm then scalar ring order")

            def load_x(s, cb, half):
                r = slice(0, ROWS_A) if half == 0 else slice(ROWS_A, HP)
                nc.sync.dma_start(xs[(s, cb)][:, r, :],
                                  xp.ap()[s, cb * P:(cb + 1) * P, r, :])

            nc.sync.dma_start(cst_t[:], cst.ap())
            load_x(0, 0, 0)
            nc.sync.dma_start(ut_t[1][:, 0:KA, :], ut.ap()[P:2 * P, 0:KA])
            load_x(0, 1, 0)
            nc.sync.dma_start(ut_t[1][:, KA:KK, :], ut.ap()[P:2 * P, KA:KK])
            nc.sync.dma_start(w2_t[:], w2.ap())
            load_x(0, 0, 1)
            load_x(0, 1, 1)

            # ---------- style (PE): [P(cin), B_LOC] per cin block ----------
            style_col = []
            style2 = []
            style_sq_i = []
            for it in range(NI):
                ps = psum.tile([P, B_LOC], F32, name=f"styp{it}", tag=f"kh{2 + it}")
                for sb in range(NS):
                    nc.tensor.matmul(
                        ps[:], swt_t[:, sb, it * P:(it + 1) * P],
                        swt_t[:, sb, CIN:CIN + B_LOC],
                        start=(sb == 0), stop=(sb == NS - 1),
                    )
                sc = consts.tile([P, B_LOC], F32, name=f"stc{it}", tag=f"stc{it}")
                nc.scalar.activation(sc[:], ps[:], AF.Identity,
                                     bias=style_b_ap(it))
                s2 = consts.tile([P, B_LOC], F32, name=f"st2{it}", tag=f"st2{it}")
                style_sq_i.append(nc.vector.tensor_mul(s2[:], sc[:], sc[:]))
                style_col.append(sc)
                style2.append(s2)

            # ---------- per-sample modulated U weights (DVE) ----------
            um_i = {}

            def make_um(s, it):
                t = um[(s, it)]
                for k0, k1 in ((0, KA), (KA, KK)):
                    um_i.setdefault(s, []).append(nc.vector.tensor_scalar_mul(
                        t[:, k0:k1, :], ut_t[it][:, k0:k1, :],
                        style_col[it][:, s:s + 1]))

            # ---------- V input transform (DVE) ----------
            # V[kh][:, m, :] for tile-rows m in [m0, m0+n):
            #   d_k = xp rows 2m+k  ->  V0=d0-d2 V1=d1+d2 V2=d2-d1 V3=d1-d3
            v_i = {}

            def make_v(s, cb, m0, n):
                x_t = xs[(s, cb)]
                v = vt[(s, cb)]

                def d(k):
                    a = 2 * m0 + k
                    return x_t[:, a:a + 2 * n - 1:2, :]

                ms = slice(m0, m0 + n)
                lst = v_i.setdefault(s, [])
                lst.append(nc.vector.tensor_sub(v[:, 0, ms, :], d(0), d(2)))
                lst.append(nc.vector.tensor_add(v[:, 1, ms, :], d(1), d(2)))
                lst.append(nc.vector.tensor_sub(v[:, 2, ms, :], d(2), d(1)))
                lst.append(nc.vector.tensor_sub(v[:, 3, ms, :], d(1), d(3)))

            # sample 0, DVE program order by data readiness: um(it0) first
            # (ut0a+swt land first), then all cb0 V (the it0 matmuls read
            # only cb0), then cb1 V, then um(it1) (gated on ut1)
            make_um(0, 0)
            for m0 in (0, 8):
                make_v(0, 0, m0, 8)
            for m0 in (0, 8):
                make_v(0, 1, m0, 8)
            make_um(0, 1)

            # ---------- main conv block: 24 matmuls per chunk ----------
            def alloc_pcs(s, ot, tr0):
                return [psum.tile([P, 8, W], F32, name=f"pc{s}{ot}{tr0}_{k}",
                                  tag=f"kh{k}")
                        for k in range(KH)]

            def mm_half(s, ot, tr0, trsz, it, pcs):
                for kh in range(KH):
                    for kx in range(KX):
                        lhsT = um[(s, it)][:, kh * KX + kx,
                                           ot * P:(ot + 1) * P]
                        rhs = vt[(s, it)][:, kh, tr0:tr0 + trsz, kx:kx + W]
                        nc.tensor.matmul(
                            pcs[kh][:, 0:trsz, :], lhsT, rhs,
                            start=(it == 0 and kx == 0),
                            stop=(it == NI - 1 and kx == KX - 1),
                        )

            def mm_block(s, ot, tr0, trsz):
                pcs = alloc_pcs(s, ot, tr0)
                for it in range(NI):
                    mm_half(s, ot, tr0, trsz, it, pcs)
                return pcs

            def out_block(s, ot, tr0, trsz, pcs):
                z = slice(0, trsz)
                m1s = temps.tile([P, 8, W], F32, name=f"m1s{s}{ot}{tr0}",
                                 tag="m1s")
                nc.scalar.copy(m1s[:, z, :], pcs[1][:, z, :])
                t0 = temps.tile([P, 8, W], F32, name=f"t0{s}{ot}{tr0}", tag="t0")
                t1 = temps.tile([P, 8, W], F32, name=f"t1{s}{ot}{tr0}", tag="t1")
                y0 = temps.tile([P, 8, W], BF16, name=f"y0{s}{ot}{tr0}", tag="y0")
                y1 = temps.tile([P, 8, W], BF16, name=f"y1{s}{ot}{tr0}", tag="y1")
                nc.vector.tensor_add(t0[:, z, :], m1s[:, z, :], pcs[0][:, z, :])
                nc.vector.tensor_add(y0[:, z, :], t0[:, z, :], pcs[2][:, z, :])
                nc.vector.tensor_sub(t1[:, z, :], m1s[:, z, :], pcs[2][:, z, :])
                nc.vector.tensor_sub(y1[:, z, :], t1[:, z, :], pcs[3][:, z, :])
                oh = out_pool.tile([P, 2, 8, W], BF16, name=f"oh{s}{ot}{tr0}",
                                   tag="oh")
                for pl, yy in ((0, y0), (1, y1)):
                    nc.scalar.activation(
                        oh[:, pl, z, :], yy[:, z, :], AF.Identity,
                        bias=bias_ap(ot), scale=winv[ot][:, s:s + 1],
                    )
                nc.sync.dma_start(
                    out.ap()[s, ot * P:(ot + 1) * P, :, tr0:tr0 + trsz, :],
                    oh[:, :, z, :])

            # first two conv chunks split by cin-block: the it=0 matmuls
            # only need ut0 + x h0, so PE starts while ut1 still loads
            pcs_c0 = alloc_pcs(0, 0, 0)
            pcs_c1 = alloc_pcs(0, 0, 8)
            mm_half(0, 0, 0, 8, 0, pcs_c0)
            mm_half(0, 0, 8, 8, 0, pcs_c1)
            mm_half(0, 0, 0, 8, 1, pcs_c0)
            mm_half(0, 0, 8, 8, 1, pcs_c1)

            # ---------- demod path (behind the first block) ----------
            # sigma[o_part, b] = sum_i W2T[i,o] * style2[i,b]  (PE, f32)
            winv = []
            for ot in range(NO):
                ps = psum.tile([P, B_LOC], F32, name=f"sig{ot}", tag=f"kh{ot}")
                for it in range(NI):
                    nc.tensor.matmul(
                        ps[:], w2_t[:, it, ot * P:(ot + 1) * P], style2[it][:],
                        start=(it == 0), stop=(it == NI - 1),
                    )
                wstd = consts.tile([P, B_LOC], F32, name=f"wstd{ot}",
                                   tag=f"wstd{ot}")
                nc.scalar.activation(wstd[:], ps[:], AF.Sqrt, bias=eps_ap)
                wi = consts.tile([P, B_LOC], F32, name=f"winv{ot}",
                                 tag=f"winv{ot}")
                nc.vector.reciprocal(wi[:], wstd[:])
                winv.append(wi)

            # rest of sample-0 V transform (tile-rows 16..32)
            for cb in range(NI):
                make_v(0, cb, 16, 16)

            # ---------- rest of the schedule ----------
            out_block(0, 0, 0, 8, pcs_c0)
            # sample-1 x loads queue behind the first out stores
            for half in range(2):
                for cb in range(NI):
                    load_x(1, cb, half)
            out_block(0, 0, 8, 8, pcs_c1)
            for tr0 in (16, 24):
                out_block(0, 0, tr0, 8, mm_block(0, 0, tr0, 8))
            # sample-1 weight mod + V transform, off the critical path
            make_um(1, 0)
            make_um(1, 1)
            for m0 in (0, 16):
                for cb in range(NI):
                    make_v(1, cb, m0, 16)
            for tr0 in (0, 8, 16, 24):
                out_block(0, 1, tr0, 8, mm_block(0, 1, tr0, 8))
            for tr0 in (0, 8, 16, 24):
                out_block(1, 0, tr0, 8, mm_block(1, 0, tr0, 8))
            # shrinking tail so the final drain after the last matmul is short
            for tr0, trsz in ((0, 8), (8, 8), (16, 8), (24, 4), (28, 2), (30, 2)):
                out_block(1, 1, tr0, trsz, mm_block(1, 1, tr0, trsz))
    return nc


_CACHED_NC = None


def _get_nc():
    global _CACHED_NC
    if _CACHED_NC is None:
        nc = bacc.Bacc("TRN2", target_bir_lowering=False, debug=False,
                       num_devices=N_CORES)
        build_conv2dmod(nc)
        nc.compile()
        _CACHED_NC = nc
    return _CACHED_NC


def kernel(x, y, weights, bias, style_w, style_b, _trace=False):
    x = np.asarray(x, dtype=np.float32)
    y = np.asarray(y, dtype=np.float32)
    weights = np.asarray(weights, dtype=np.float32)
    bias = np.asarray(bias, dtype=np.float32)
    style_w = np.asarray(style_w, dtype=np.float32)
    style_b = np.asarray(style_b, dtype=np.float32)

    # host-side layout packing (see module docstring)
    G = np.array([[1, 0, 0], [0.5, 0.5, 0.5], [0.5, -0.5, 0.5], [0, 0, 1]],
                 np.float32)
    # U[i, kh, kx, o] = sum_ky G[kh,ky] w[o,i,ky,kx]  -> [Cin, 12, Cout] bf16
    U = np.einsum("hk,oikx->ihxo", G, weights)
    ut = np.ascontiguousarray(U.reshape(CIN, KK, COUT)).astype(ml_dtypes.bfloat16)
    # W2[i,o] = sum_kk w[o,i,kk]^2  -> [P, NI, COUT] f32
    W2 = np.einsum("oikl->io", weights.astype(np.float64) ** 2).astype(np.float32)
    w2 = np.ascontiguousarray(W2.reshape(NI, P, COUT).transpose(1, 0, 2))
    swtf = style_w.T.reshape(NS, P, CIN).transpose(1, 0, 2)  # [sp, sb, i]
    ytf = y.T.reshape(NS, P, B_FULL).transpose(1, 0, 2)      # [sp, sb, b_full]
    xp = np.zeros((B_FULL, CIN, HP, WP), dtype=ml_dtypes.bfloat16)
    xp[:, :, 1:H + 1, 1:W + 1] = x.astype(ml_dtypes.bfloat16)
    cst = np.empty((P, 2 * NO + 1), dtype=np.float32)
    cst[:, 0:NO] = bias.reshape(NO, P).T
    cst[:, NO:2 * NO] = style_b.reshape(NI, P).T
    cst[:, 2 * NO] = EPS

    nc = _get_nc()
    in_maps = []
    for c in range(N_CORES):
        swt_c = np.empty((P, NS, CIN + B_LOC), dtype=ml_dtypes.bfloat16)
        swt_c[:, :, :CIN] = swtf
        swt_c[:, :, CIN:] = ytf[:, :, c * B_LOC:(c + 1) * B_LOC]
        in_maps.append({
            "xp": np.ascontiguousarray(xp[c * B_LOC:(c + 1) * B_LOC]),
            "ut": ut,
            "swt": swt_c,
            "cst": cst,
            "w2": w2,
        })
    res = bass_utils.run_bass_kernel_spmd(
        nc, in_maps, core_ids=list(range(N_CORES)), trace=_trace
    )
    # out planes: [B_LOC, COUT, 2, 32, 64] -> interleave row parity
    out = np.concatenate(
        [r["out"].transpose(0, 1, 3, 2, 4).reshape(B_LOC, COUT, H, W)
         for r in res.results], axis=0).astype(np.float32)
    if _trace:
        kernel.last_results = res
    return out


# revision 31
# speedup vs baseline: 1.0053x; 1.0053x over previous
"""Modulated Conv2D (StyleGAN2-style) Trainium2 Bass kernel.

Problem shapes (hardcoded):
  x: [16, 256, 64, 64] f32    y: [16, 512] f32
  weights: [256, 256, 3, 3]   bias: [256]
  style_w: [256, 512]         style_b: [256]
  out: [16, 256, 64, 64] f32

Formulation: 1-D Winograd F(2,3) along H + direct 3-tap conv along W,
with the per-sample style modulation folded into the weights:
  style[b,i] = y[b] @ style_w[i] + style_b[i]        (tiny PE matmul)
  U[kh,kx,i,o] = sum_ky G[kh,ky] * w[o,i,ky,kx]      (host precompute, bf16)
  um[b][i,kh*3+kx,o] = U[i,kh*3+kx,o] * style[b,i]   (DVE)
  V0 = d0-d2; V1 = d1+d2; V2 = d2-d1; V3 = d1-d3     (DVE; dk = padded x
                                                      rows 2m+k, tile-row m)
  M[kh][o,m,c] = sum_{i,kx} um[kh,kx,i,o] V[kh][i,m,c+kx]   (PE, f32 PSUM)
  out[2m+0] = (M0+M1+M2) / wstd + bias               (DVE adds + scalar act)
  out[2m+1] = (M1-M2-M3) / wstd + bias
  wstd[b,o] = sqrt(sum_i W2[i,o] * style[b,i]^2 + eps),
  W2[i,o] = sum_kk w[o,i,kk]^2                       (host precompute)

This does the conv in 24 accumulating matmuls per 8-tile-row chunk
instead of direct conv's 36 (1.5x fewer PE cycles); the Winograd
transforms run on DVE/scalar fully under the PE stream.

Host-side layout prep (layout/dtype packing + static weight transform):
  - ut = G-transformed weights [Cin, 12, Cout] bf16 (lhsT-ready).
  - w2 = per-(i,o) weight square sums f32 (demod path operand).
  - swt/yT packed bf16 so style is a [512]-contraction PE matmul.
  - x zero-padded to [66,66] bf16, matmul-ready tiles.
  - device output bf16 with even/odd output rows in separate planes;
    host interleaves + upcasts (budget 2e-2, this scheme ~4.5e-3).

Sharding: data-parallel over batch, 2 samples per core across 8 cores.

Engines: PE does style + sigma + conv matmuls; DVE squares style,
modulates U, computes the V input transform and the M->Y output
transform; Scalar copies M1 out of PSUM and applies 1/wstd + bias
(f32->bf16); scalar HW DMA ring loads weights, sync ring x + stores.
"""

import numpy as np
import ml_dtypes

import concourse.bass as bass
import concourse.tile as tile
from concourse import bacc, mybir
from concourse import bass_utils

EPS = 1e-8
P = 128
B_LOC = 2          # samples per core
B_FULL = 16
CIN, COUT = 256, 256
NI, NO = CIN // P, COUT // P   # 2, 2
S = 512
NS = S // P        # 4 style contraction blocks
KH, KX = 4, 3      # winograd points along H, direct taps along W
KK = KH * KX       # 12 lhsT planes
KA = 6             # first kk-chunk of the ut loads
H = W = 64
HP, WP = H + 2, W + 2  # zero-padded image
TR = H // 2        # 32 winograd tile-rows
N_CORES = 8
ROWS_A = 34        # rows in the first half of each x tile load

F32 = mybir.dt.float32
BF16 = mybir.dt.bfloat16
AF = mybir.ActivationFunctionType


def _chain(instrs, reason):
    """Force program order on one engine (guides the tile scheduler)."""
    for a, b in zip(instrs[1:], instrs[:-1]):
        bass._add_dep_helper(a.ins, b.ins, sync=False, reason=reason)


def build_conv2dmod(nc):
    xp = nc.dram_tensor("xp", [B_LOC, CIN, HP, WP], BF16, kind="ExternalInput")
    ut = nc.dram_tensor("ut", [CIN, KK, COUT], BF16, kind="ExternalInput")
    # host-prepacked [sp, sb, i+b]: style_w.T columns then y.T columns
    swt = nc.dram_tensor("swt", [P, NS, CIN + B_LOC], BF16, kind="ExternalInput")
    # host-prepacked [128, 5] f32: bias cols (2), style_b cols (2), eps (1)
    cst = nc.dram_tensor("cst", [P, 2 * NO + 1], F32, kind="ExternalInput")
    # host-prepacked W2T [i_part, it, o] f32
    w2 = nc.dram_tensor("w2", [P, NI, COUT], F32, kind="ExternalInput")
    # even/odd output rows in separate planes; host interleaves
    out = nc.dram_tensor("out", [B_LOC, COUT, 2, TR, W], BF16,
                         kind="ExternalOutput")

    with tile.TileContext(nc) as tc:
        with (
            tc.tile_pool(name="consts", bufs=1) as consts,
            tc.tile_pool(name="temps", bufs=2) as temps,
            tc.tile_pool(name="um_pool", bufs=1) as um_pool,
            tc.tile_pool(name="xs_pool", bufs=1) as xs_pool,
            tc.tile_pool(name="v_pool", bufs=1) as v_pool,
            tc.tile_pool(name="out_pool", bufs=3) as out_pool,
            tc.tile_pool(name="psum", bufs=2, space="PSUM") as psum,
        ):
            # ---------------- tiles ----------------
            swt_t = consts.tile([P, NS, CIN + B_LOC], BF16)
            ut_t = [consts.tile([P, KK, COUT], BF16, name=f"ut{i}", tag=f"ut{i}")
                    for i in range(NI)]
            w2_t = consts.tile([P, NI, COUT], F32)
            cst_t = consts.tile([P, 2 * NO + 1], F32)
            xs = {}
            vt = {}
            for s in range(B_LOC):
                for cb in range(NI):
                    xs[(s, cb)] = xs_pool.tile(
                        [P, HP, WP], BF16, name=f"xs{s}_{cb}", tag=f"xs{s}_{cb}")
                    vt[(s, cb)] = v_pool.tile(
                        [P, KH, TR, WP], BF16, name=f"vt{s}_{cb}",
                        tag=f"vt{s}_{cb}")
            um = {}
            for s in range(B_LOC):
                for it in range(NI):
                    um[(s, it)] = um_pool.tile(
                        [P, KK, COUT], BF16, name=f"um{s}_{it}",
                        tag=f"um{s}_{it}")

            def bias_ap(ot):
                return cst_t[:, ot:ot + 1]

            def style_b_ap(it):
                return cst_t[:, NO + it:NO + it + 1]

            eps_ap = cst_t[:, 2 * NO:2 * NO + 1]

            # pre-warm the ACT function table that Sqrt lives in; must be
            # dependency-free so it runs (and the async table load fires)
            # before anything else on the scalar engine
            warm_src = consts.tile([P, 1], F32)
            nc.gpsimd.memset(warm_src[:], EPS)
            lafs_warm = consts.tile([P, 1], F32)
            warm_i = nc.scalar.activation(lafs_warm[:], warm_src[:], AF.Sqrt)

            # PE HAM warm-up: ~3.5us of zeroed junk matmuls during the DMA
            # prologue so the PE clock gate is already at 8/8 (2.4 GHz) when
            # the real conv stream starts. The junk psum tiles borrow the
            # style tags ahead of their first real use so the conv chunks'
            # kh0/kh1 rotation parity is untouched.
            jw = consts.tile([P, P], BF16)
            jx = consts.tile([P, 8, W], BF16)
            nc.gpsimd.memset(jw[:], 0.0)
            nc.gpsimd.memset(jx[:], 0.0)
            for r in range(19):
                pj = psum.tile([P, 8, W], F32, name=f"jp{r}",
                               tag=f"kh{2 + r % 2}")
                nc.tensor.matmul(pj[:], jw[:], jx[:], start=True, stop=True)

            # ------------- DMA rings, ordered by when they gate compute -------
            # the two HWDGE rings share ~330 GB/s of HBM bandwidth; order
            # both by when the bytes gate compute (it0 weights + x h0 first)
            scalar_ring = [
                nc.scalar.dma_start(ut_t[0][:, 0:KA, :], ut.ap()[0:P, 0:KA]),
                nc.scalar.dma_start(swt_t[:], swt.ap()),
                nc.scalar.dma_start(ut_t[0][:, KA:KK, :], ut.ap()[0:P, KA:KK]),
            ]
            _chain([warm_i] + scalar_ring, "warm then scalar ring order")

            def load_x(s, cb, half):
                r = slice(0, ROWS_A) if half == 0 else slice(ROWS_A, HP)
                nc.sync.dma_start(xs[(s, cb)][:, r, :],
                                  xp.ap()[s, cb * P:(cb + 1) * P, r, :])

            nc.sync.dma_start(cst_t[:], cst.ap())
            load_x(0, 0, 0)
            nc.sync.dma_start(ut_t[1][:, 0:KA, :], ut.ap()[P:2 * P, 0:KA])
            load_x(0, 1, 0)
            nc.sync.dma_start(ut_t[1][:, KA:KK, :], ut.ap()[P:2 * P, KA:KK])
            nc.sync.dma_start(w2_t[:], w2.ap())
            load_x(0, 0, 1)
            load_x(0, 1, 1)

            # ---------- style (PE): [P(cin), B_LOC] per cin block ----------
            style_col = []
            style2 = []
            style_sq_i = []
            for it in range(NI):
                ps = psum.tile([P, B_LOC], F32, name=f"styp{it}", tag=f"kh{2 + it}")
                for sb in range(NS):
                    nc.tensor.matmul(
                        ps[:], swt_t[:, sb, it * P:(it + 1) * P],
                        swt_t[:, sb, CIN:CIN + B_LOC],
                        start=(sb == 0), stop=(sb == NS - 1),
                    )
                sc = consts.tile([P, B_LOC], F32, name=f"stc{it}", tag=f"stc{it}")
                nc.scalar.activation(sc[:], ps[:], AF.Identity,
                                     bias=style_b_ap(it))
                s2 = consts.tile([P, B_LOC], F32, name=f"st2{it}", tag=f"st2{it}")
                style_sq_i.append(nc.vector.tensor_mul(s2[:], sc[:], sc[:]))
                style_col.append(sc)
                style2.append(s2)

            # ---------- per-sample modulated U weights (DVE) ----------
            um_i = {}

            def make_um(s, it):
                t = um[(s, it)]
                for k0, k1 in ((0, KA), (KA, KK)):
                    um_i.setdefault(s, []).append(nc.vector.tensor_scalar_mul(
                        t[:, k0:k1, :], ut_t[it][:, k0:k1, :],
                        style_col[it][:, s:s + 1]))

            # ---------- V input transform (DVE) ----------
            # V[kh][:, m, :] for tile-rows m in [m0, m0+n):
            #   d_k = xp rows 2m+k  ->  V0=d0-d2 V1=d1+d2 V2=d2-d1 V3=d1-d3
            v_i = {}

            def make_v(s, cb, m0, n):
                x_t = xs[(s, cb)]
                v = vt[(s, cb)]

                def d(k):
                    a = 2 * m0 + k
                    return x_t[:, a:a + 2 * n - 1:2, :]

                ms = slice(m0, m0 + n)
                lst = v_i.setdefault(s, [])
                lst.append(nc.vector.tensor_sub(v[:, 0, ms, :], d(0), d(2)))
                lst.append(nc.vector.tensor_add(v[:, 1, ms, :], d(1), d(2)))
                lst.append(nc.vector.tensor_sub(v[:, 2, ms, :], d(2), d(1)))
                lst.append(nc.vector.tensor_sub(v[:, 3, ms, :], d(1), d(3)))

            # sample 0, DVE program order by data readiness: um(it0) first
            # (ut0a+swt land first), then all cb0 V (the it0 matmuls read
            # only cb0), then cb1 V, then um(it1) (gated on ut1)
            make_um(0, 0)
            for m0 in (0, 8):
                make_v(0, 0, m0, 8)
            for m0 in (0, 8):
                make_v(0, 1, m0, 8)
            make_um(0, 1)

            # ---------- main conv block: 24 matmuls per chunk ----------
            def alloc_pcs(s, ot, tr0):
                return [psum.tile([P, 8, W], F32, name=f"pc{s}{ot}{tr0}_{k}",
                                  tag=f"kh{k}")
                        for k in range(KH)]

            def mm_half(s, ot, tr0, trsz, it, pcs):
                for kh in range(KH):
                    for kx in range(KX):
                        lhsT = um[(s, it)][:, kh * KX + kx,
                                           ot * P:(ot + 1) * P]
                        rhs = vt[(s, it)][:, kh, tr0:tr0 + trsz, kx:kx + W]
                        nc.tensor.matmul(
                            pcs[kh][:, 0:trsz, :], lhsT, rhs,
                            start=(it == 0 and kx == 0),
                            stop=(it == NI - 1 and kx == KX - 1),
                        )

            def mm_block(s, ot, tr0, trsz):
                pcs = alloc_pcs(s, ot, tr0)
                for it in range(NI):
                    mm_half(s, ot, tr0, trsz, it, pcs)
                return pcs

            def out_block(s, ot, tr0, trsz, pcs):
                z = slice(0, trsz)
                m1s = temps.tile([P, 8, W], F32, name=f"m1s{s}{ot}{tr0}",
                                 tag="m1s")
                nc.scalar.copy(m1s[:, z, :], pcs[1][:, z, :])
                t0 = temps.tile([P, 8, W], F32, name=f"t0{s}{ot}{tr0}", tag="t0")
                t1 = temps.tile([P, 8, W], F32, name=f"t1{s}{ot}{tr0}", tag="t1")
                y0 = temps.tile([P, 8, W], BF16, name=f"y0{s}{ot}{tr0}", tag="y0")
                y1 = temps.tile([P, 8, W], BF16, name=f"y1{s}{ot}{tr0}", tag="y1")
                nc.vector.tensor_add(t0[:, z, :], m1s[:, z, :], pcs[0][:, z, :])
                nc.vector.tensor_add(y0[:, z, :], t0[:, z, :], pcs[2][:, z, :])
                nc.vector.tensor_sub(t1[:, z, :], m1s[:, z, :], pcs[2][:, z, :])
                nc.vector.tensor_sub(y1[:, z, :], t1[:, z, :], pcs[3][:, z, :])
                oh = out_pool.tile([P, 2, 8, W], BF16, name=f"oh{s}{ot}{tr0}",
                                   tag="oh")
                for pl, yy in ((0, y0), (1, y1)):
                    nc.scalar.activation(
                        oh[:, pl, z, :], yy[:, z, :], AF.Identity,
                        bias=bias_ap(ot), scale=winv[ot][:, s:s + 1],
                    )
                nc.sync.dma_start(
                    out.ap()[s, ot * P:(ot + 1) * P, :, tr0:tr0 + trsz, :],
                    oh[:, :, z, :])

            # first two conv chunks split by cin-block: the it=0 matmuls
            # only need ut0 + x h0, so PE starts while ut1 still loads
            pcs_c0 = alloc_pcs(0, 0, 0)
            pcs_c1 = alloc_pcs(0, 0, 8)
            mm_half(0, 0, 0, 8, 0, pcs_c0)
            mm_half(0, 0, 8, 8, 0, pcs_c1)
            mm_half(0, 0, 0, 8, 1, pcs_c0)
            mm_half(0, 0, 8, 8, 1, pcs_c1)

            # ---------- demod path (behind the first block) ----------
            # sigma[o_part, b] = sum_i W2T[i,o] * style2[i,b]  (PE, f32)
            winv = []
            for ot in range(NO):
                ps = psum.tile([P, B_LOC], F32, name=f"sig{ot}", tag=f"kh{ot}")
                for it in range(NI):
                    nc.tensor.matmul(
                        ps[:], w2_t[:, it, ot * P:(ot + 1) * P], style2[it][:],
                        start=(it == 0), stop=(it == NI - 1),
                    )
                wstd = consts.tile([P, B_LOC], F32, name=f"wstd{ot}",
                                   tag=f"wstd{ot}")
                nc.scalar.activation(wstd[:], ps[:], AF.Sqrt, bias=eps_ap)
                wi = consts.tile([P, B_LOC], F32, name=f"winv{ot}",
                                 tag=f"winv{ot}")
                nc.vector.reciprocal(wi[:], wstd[:])
                winv.append(wi)

            # rest of sample-0 V transform (tile-rows 16..32)
            for cb in range(NI):
                make_v(0, cb, 16, 16)

            # ---------- rest of the schedule ----------
            out_block(0, 0, 0, 8, pcs_c0)
            # sample-1 x loads queue behind the first out stores
            for half in range(2):
                for cb in range(NI):
                    load_x(1, cb, half)
            out_block(0, 0, 8, 8, pcs_c1)
            for tr0 in (16, 24):
                out_block(0, 0, tr0, 8, mm_block(0, 0, tr0, 8))
            # sample-1 weight mod + V transform, off the critical path
            make_um(1, 0)
            make_um(1, 1)
            for m0 in (0, 16):
                for cb in range(NI):
                    make_v(1, cb, m0, 16)
            for tr0 in (0, 8, 16, 24):
                out_block(0, 1, tr0, 8, mm_block(0, 1, tr0, 8))
            for tr0 in (0, 8, 16, 24):
                out_block(1, 0, tr0, 8, mm_block(1, 0, tr0, 8))
            # shrinking tail so the final drain after the last matmul is short
            for tr0, trsz in ((0, 8), (8, 8), (16, 8), (24, 4), (28, 2), (30, 2)):
                out_block(1, 1, tr0, trsz, mm_block(1, 1, tr0, trsz))
    return nc


_CACHED_NC = None


def _get_nc():
    global _CACHED_NC
    if _CACHED_NC is None:
        nc = bacc.Bacc("TRN2", target_bir_lowering=False, debug=False,
                       num_devices=N_CORES)
        build_conv2dmod(nc)
        nc.compile()
        _CACHED_NC = nc
    return _CACHED_NC


def kernel(x, y, weights, bias, style_w, style_b, _trace=False):
    x = np.asarray(x, dtype=np.float32)
    y = np.asarray(y, dtype=np.float32)
    weights = np.asarray(weights, dtype=np.float32)
    bias = np.asarray(bias, dtype=np.float32)
    style_w = np.asarray(style_w, dtype=np.float32)
    style_b = np.asarray(style_b, dtype=np.float32)

    # host-side layout packing (see module docstring)
    G = np.array([[1, 0, 0], [0.5, 0.5, 0.5], [0.5, -0.5, 0.5], [0, 0, 1]],
                 np.float32)
    # U[i, kh, kx, o] = sum_ky G[kh,ky] w[o,i,ky,kx]  -> [Cin, 12, Cout] bf16
    U = np.einsum("hk,oikx->ihxo", G, weights)
    ut = np.ascontiguousarray(U.reshape(CIN, KK, COUT)).astype(ml_dtypes.bfloat16)
    # W2[i,o] = sum_kk w[o,i,kk]^2  -> [P, NI, COUT] f32
    W2 = np.einsum("oikl->io", weights.astype(np.float64) ** 2).astype(np.float32)
    w2 = np.ascontiguousarray(W2.reshape(NI, P, COUT).transpose(1, 0, 2))
    swtf = style_w.T.reshape(NS, P, CIN).transpose(1, 0, 2)  # [sp, sb, i]
    ytf = y.T.reshape(NS, P, B_FULL).transpose(1, 0, 2)      # [sp, sb, b_full]
    xp = np.zeros((B_FULL, CIN, HP, WP), dtype=ml_dtypes.bfloat16)
    xp[:, :, 1:H + 1, 1:W + 1] = x.astype(ml_dtypes.bfloat16)
    cst = np.empty((P, 2 * NO + 1), dtype=np.float32)
    cst[:, 0:NO] = bias.reshape(NO, P).T
    cst[:, NO:2 * NO] = style_b.reshape(NI, P).T
    cst[:, 2 * NO] = EPS

    nc = _get_nc()
    in_maps = []
    for c in range(N_CORES):
        swt_c = np.empty((P, NS, CIN + B_LOC), dtype=ml_dtypes.bfloat16)
        swt_c[:, :, :CIN] = swtf
        swt_c[:, :, CIN:] = ytf[:, :, c * B_LOC:(c + 1) * B_LOC]
        in_maps.append({
            "xp": np.ascontiguousarray(xp[c * B_LOC:(c + 1) * B_LOC]),
            "ut": ut,
            "swt": swt_c,
            "cst": cst,
            "w2": w2,
        })
    res = bass_utils.run_bass_kernel_spmd(
        nc, in_maps, core_ids=list(range(N_CORES)), trace=_trace
    )
    # out planes: [B_LOC, COUT, 2, 32, 64] -> interleave row parity
    out = np.concatenate(
        [r["out"].transpose(0, 1, 3, 2, 4).reshape(B_LOC, COUT, H, W)
         for r in res.results], axis=0).astype(np.float32)
    if _trace:
        kernel.last_results = res
    return out
